# revision 16
# baseline (speedup 1.0000x reference)
"""ColorUnpool (gather + segment-max + relu) as an 8-core Trainium2 Bass kernel.

Reference semantics:
    out = zeros([200000, 256]);  out[center_idx] = feat            # centers
    seg = segment_max(feat[edge_src], edge_dst)                    # edges
    out[r] = max(seg[r], 0) for rows r with >= 1 incoming edge

edge_dst only hits rows [50000, 200000) and center_idx only [0, 50000), so
the two regions are disjoint.  The center region is a pure host-side copy of
the input (no compute); the device computes the edge region only.

Device strategy (per core, dst rows split 8 ways -> 18750 rows/core):
  * Rows are degree-sorted (desc) and packed into 147 tiles of 128 rows.
    Column layout is round-major: round 0 holds one column per tile (edge 0
    of every row, ZID pad for deg-0 rows); round j>=1 holds a column for
    each tile whose max degree exceeds j (a prefix, since tiles are sorted).
  * The feat table is compacted per core to its ~31.6k distinct src rows
    (< 32768), so gather indices fit in int16 and the gather runs as
    1024-index `dma_gather` instructions (the HW cap) round-robined over
    all 4 SWDGE queues -- descriptor generation for different queues runs
    concurrently on the Q7 cores, which quarters the ~7.7ns/row software
    DGE cost that serialized the old per-column indirect-DMA design.
  * Round 0 gathers straight into the accumulator; rounds j>=1 gather into
    rotating SBUF chunks and fold in with fused DVE ops
    acc = max(max(acc, 0), g)  (scalar_tensor_tensor), which also bakes in
    the final relu.  Tiles only touched by round 0 get an Activation-engine
    relu instead.  Finished tile ranges are written back to DRAM as soon as
    their last round completes, overlapping the output DMA with the
    remaining gathers.
  * feat is bf16 on device (rel err ~4e-3 << 2e-2 gate); the host
    un-permutes rows and upcasts to f32.
"""

import sys
import types

import numpy as np
import ml_dtypes

sys.path.insert(0, "/opt/trn_rl_repo")

N_NODES = 200000
N_CENTERS = 50000
FEAT = 256
NCORES = 8
P = 128

R_EDGE = N_NODES - N_CENTERS          # 150000 edge-target rows
RC = R_EDGE // NCORES                 # 18750 edge rows per core
TILES = (RC + P - 1) // P             # 147 tiles of 128 rows
NPOS = TILES * P                      # 18816 padded row slots
TBL = 32768                           # per-core compact feat table rows
ZID = TBL - 1                         # zero row id (table is zero-padded)
G = 8                                 # gather chunk width (cols); HW caps a
                                      # single dma_gather at 1024 indices
WMIN = 8                              # min writeback width (tiles)


def _install_profile_hook():
    """Provide antenv.axon_hooks (missing on this image) so that
    run_bass_kernel_spmd(trace=True) can profile via the axon .so."""
    try:
        import antenv
        if "antenv.axon_hooks" in sys.modules:
            return
        from trn_agent_boot.trn_boot import _ntff_profile_via_ctypes
        mod = types.ModuleType("antenv.axon_hooks")
        hook = _ntff_profile_via_ctypes("/opt/axon/libaxon_pjrt.so")
        mod.get_axon_ntff_profile_hook = lambda: hook
        mod.set_axon_ntff_profile_hook = lambda h: None
        sys.modules["antenv.axon_hooks"] = mod
        antenv.axon_hooks = mod
    except Exception:
        pass


def _build_plan(edge_src, edge_dst, feat):
    """Host preprocessing.

    Returns (T, bases, C, tables, idx_planes, orders):
      T          = per-round union active-tile counts, T[0] == TILES
      bases      = column base per round
      C          = total columns
      tables     = per-core compact bf16 feat tables [TBL, FEAT]
      idx_planes = per-core int16 idx planes [P, C*8] (x8 Q7 replication)
      orders     = per-core position->local-row permutation [RC]
    """
    edge_src = np.asarray(edge_src, np.int64)
    edge_dst = np.asarray(edge_dst, np.int64)
    local_dst = edge_dst - N_CENTERS
    assert local_dst.min() >= 0 and local_dst.max() < R_EDGE
    core_of = local_dst // RC

    percore = []
    for c in range(NCORES):
        m = core_of == c
        ld = (local_dst[m] % RC).astype(np.int64)
        ss = edge_src[m].astype(np.int64)
        deg = np.bincount(ld, minlength=RC)
        order = np.argsort(-deg, kind="stable")          # rows desc by degree
        eo = np.argsort(ld, kind="stable")
        ss_sorted = ss[eo]                               # CSR values
        starts = np.concatenate([[0], np.cumsum(deg)[:-1]])
        uniq, inv = np.unique(ss_sorted, return_inverse=True)
        assert len(uniq) < TBL, f"core {c}: {len(uniq)} distinct srcs > int16"
        ssc = inv.astype(np.int64)                       # compact CSR values
        deg_sorted = deg[order]
        d_tile = deg_sorted[np.arange(TILES) * P]        # per-tile max degree
        percore.append(dict(deg=deg, order=order, ssc=ssc, starts=starts,
                            d_tile=d_tile, uniq=uniq))

    maxd = max(max(int(pc["d_tile"][0]), 1) for pc in percore)
    T = [TILES]                                          # round 0: all tiles
    for j in range(1, maxd):
        T.append(max(int((pc["d_tile"] > j).sum()) for pc in percore))
    bases = np.concatenate([[0], np.cumsum(T)[:-1]]).astype(int)
    C = int(np.sum(T))

    tables, idx_planes, orders = [], [], []
    for pc in percore:
        order_padded = np.full(NPOS, -1, np.int64)
        order_padded[:RC] = pc["order"]
        deg, starts, ssc = pc["deg"], pc["starts"], pc["ssc"]
        vals = np.full(C * P, ZID, np.int64)
        for j in range(maxd):
            qpos = np.arange(T[j] * P)
            r = order_padded[qpos]
            rs = np.where(r >= 0, r, 0)
            has = (r >= 0) & (deg[rs] > j)
            v = np.where(has, ssc[np.minimum(starts[rs] + j, len(ssc) - 1)],
                         ZID)
            vals[bases[j] * P: bases[j] * P + T[j] * P] = v
        # idx position g lives at [g%16, g//16], replicated x8 for Q7 cores
        plane16 = vals.astype(np.int16).reshape(C * 8, 16).T
        idx_planes.append(np.ascontiguousarray(np.tile(plane16, (8, 1))))
        tbl = np.zeros((TBL, FEAT), ml_dtypes.bfloat16)
        tbl[:len(pc["uniq"])] = feat[pc["uniq"]].astype(ml_dtypes.bfloat16)
        tables.append(tbl)
        orders.append(pc["order"])
    return T, bases, C, tables, idx_planes, orders


def _build_bass(T, bases, C):
    import concourse.bacc as bacc
    import concourse.mybir as mybir
    import concourse.tile as tile

    maxd = len(T)
    nc = bacc.Bacc("TRN2", target_bir_lowering=False, debug=False,
                   num_devices=NCORES, num_swdge_queues=4)
    t_feat = nc.dram_tensor("feat_tbl", [TBL, FEAT], mybir.dt.bfloat16,
                            kind="ExternalInput")
    t_idx = nc.dram_tensor("idxs", [P, C * 8], mybir.dt.int16,
                           kind="ExternalInput")
    t_oe = nc.dram_tensor("out_edge", [P, TILES, FEAT], mybir.dt.bfloat16,
                          kind="ExternalOutput")

    mx = mybir.AluOpType.max
    relu = mybir.ActivationFunctionType.Relu

    # G-column chunks, split at the round-0 boundary (those go straight
    # into the accumulator)
    chunks = []
    s = 0
    while s < C:
        e = min(s + G, TILES if s < TILES else C)
        chunks.append((s, e))
        s = e



    with tile.TileContext(nc) as tc:
        with tc.tile_pool(name="idxp", bufs=1) as idxp, \
             tc.tile_pool(name="accp", bufs=1) as accp, \
             tc.tile_pool(name="gp", bufs=8) as gp:
            idx = idxp.tile([P, C * 8], mybir.dt.int16)
            # dummy 16-idx gather with no data deps: triggers the Q7 mlp
            # library IRAM load (~8us) during the preamble/idx load instead
            # of stalling the first real gather
            idxw = idxp.tile([P, 1], mybir.dt.int16)
            nc.gpsimd.memset(idxw[:], 0)
            warm = idxp.tile([P, 1, FEAT], mybir.dt.bfloat16)
            nc.gpsimd.dma_gather(warm[:], t_feat[:], idxw[:], 16, 16, FEAT,
                                 queue_num=0)
            nc.sync.dma_start(out=idx[:], in_=t_idx[:])
            acc = accp.tile([P, TILES, FEAT], mybir.dt.bfloat16)

            pend = []          # pending finalized tile ranges [lo, hi)

            def add_final(lo, hi, force=False):
                if lo < hi:
                    if pend and pend[-1][1] == lo:
                        pend[-1] = (pend[-1][0], hi)
                    elif pend and pend[-1][0] == hi:
                        pend[-1] = (lo, pend[-1][1])
                    else:
                        pend.append((lo, hi))
                keep = []
                for lo, hi in pend:
                    if hi - lo >= WMIN or force:
                        nc.sync.dma_start(out=t_oe[:, lo:hi, :],
                                          in_=acc[:, lo:hi, :])
                    else:
                        keep.append((lo, hi))
                pend[:] = keep

            for k, (cs, ce) in enumerate(chunks):
                w = ce - cs
                if ce <= TILES:                          # round 0: direct
                    gout = acc[:, cs:ce, :]
                else:
                    g = gp.tile([P, G, FEAT], mybir.dt.bfloat16, tag="g")
                    gout = g[:, :w, :]
                nc.gpsimd.dma_gather(gout, t_feat[:], idx[:, cs * 8:ce * 8],
                                     w * P, w * P, FEAT,
                                     queue_num=(k + 1) % 4)
                # per-chunk: reduce, then finalize the tiles whose last
                # round this chunk completes (spreads writes evenly)
                for j in range(maxd):
                    a = max(cs, int(bases[j]))
                    b = min(ce, int(bases[j]) + T[j])
                    if a >= b:
                        continue
                    nxt = T[j + 1] if j + 1 < maxd else 0
                    if j == 0:
                        # deg<=1 tiles: relu never fused -> Act engine
                        lo = max(a, nxt)
                        if lo < b:
                            nc.scalar.activation(acc[:, lo:b, :],
                                                 acc[:, lo:b, :], relu)
                            add_final(lo, b)
                        continue
                    t0 = a - int(bases[j])
                    L = b - a
                    if j == 1:
                        # round 1 touches every deg>=2 tile exactly once:
                        # fold the relu in; later rounds use plain max
                        nc.vector.scalar_tensor_tensor(
                            out=acc[:, t0:t0 + L, :],
                            in0=acc[:, t0:t0 + L, :], scalar=0.0,
                            in1=g[:, a - cs:b - cs, :], op0=mx, op1=mx)
                    else:
                        nc.vector.tensor_tensor(
                            out=acc[:, t0:t0 + L, :],
                            in0=acc[:, t0:t0 + L, :],
                            in1=g[:, a - cs:b - cs, :], op=mx)
                    add_final(max(t0, nxt), t0 + L)
            add_final(0, 0, force=True)
    nc.compile()
    return nc


def _unshard(results, orders, feat_centers):
    out = np.empty((N_NODES, FEAT), np.float32)
    out[:N_CENTERS] = feat_centers                       # centers: exact copy
    for c in range(NCORES):
        oe = np.asarray(results[c]["out_edge"])          # [P, TILES, FEAT]
        vals = oe.transpose(1, 0, 2).reshape(NPOS, FEAT)  # position-major
        rows = N_CENTERS + c * RC + orders[c]            # position q -> row
        out[rows] = vals[:RC].astype(np.float32)
    return out


def kernel(feat, center_idx, edge_src, edge_dst, n_nodes, _trace=False):
    assert int(n_nodes) == N_NODES
    feat = np.ascontiguousarray(np.asarray(feat, np.float32))
    center_idx = np.asarray(center_idx, np.int64)

    # centers: out[center_idx] = feat, handled fully on the host (pure copy)
    feat_centers = np.zeros((N_CENTERS, FEAT), np.float32)
    feat_centers[center_idx] = feat

    T, bases, C, tables, idx_planes, orders = _build_plan(edge_src, edge_dst,
                                                          feat)
    nc = _build_bass(T, bases, C)

    if _trace:
        _install_profile_hook()
    import concourse.bass_utils as bass_utils
    bass_utils.upload_artifacts = lambda tmpdir: f"file://{tmpdir}"
    from concourse.bass_utils import run_bass_kernel_spmd

    in_maps = [{"feat_tbl": tables[c], "idxs": idx_planes[c]}
               for c in range(NCORES)]
    kw = dict(trace=True) if _trace else {}
    res = run_bass_kernel_spmd(nc, in_maps, list(range(NCORES)), **kw)

    out = _unshard(res.results, orders, feat_centers)
    if _trace:
        return out, res
    return out
